# revision 6
# baseline (speedup 1.0000x reference)
# Trainium2 Bass kernel for nn_CombinedLoss — v4
#
# v4 = v2 (persistent rotating PSUM pool, fp8 DoubleRow sim matmuls, ACT/DVE
# exp split, rsqrt-as-exp(-ln/2)) + instruction-count reduction:
#   - elementwise prep batched over all 8 row-tiles as [128, 8, .] views with
#     stride-0 broadcast APs (one instruction instead of eight)
#   - one big DMA each for logits/embeddings
#   - phase-C class math batched over the 4 class-blocks as [128, 4, .] views
#   - per-iteration rowsum columns accumulate into one [128, 8, 4] tile,
#     reduced once
#
# Output: partials per core, reduced on host exactly like the baseline.

import numpy as np

B = 8192
C = 512
D = 256
NCORES = 8
SH = B // NCORES
T = SH // 128
ALPHA = 0.5
BETA = 0.5
GAMMA = 0.5
INV_TAU = 10.0
EPS = 1e-8
UNROLL = 128

SCH_A = 184.6649652337873 * (INV_TAU / 256.0)
SCH_B = 16248.78071298956

# number of the 8 col-chunks per row-tile handled by DVE (Schraudolph);
# chunk 0 holds the diagonal and always stays on ACT.
DVE_NJC = 2

_CACHE = {}


def _build(unroll=UNROLL):
    import concourse.bass as bass
    import concourse.mybir as mybir
    import concourse.tile as tile
    from concourse import bacc
    from concourse.masks import make_identity

    f32 = mybir.dt.float32
    f32r = mybir.dt.float32r
    bf16 = mybir.dt.bfloat16
    f8 = mybir.dt.float8e4
    i16 = mybir.dt.int16
    i32 = mybir.dt.int32
    AX = mybir.AxisListType
    OP = mybir.AluOpType
    ACT = mybir.ActivationFunctionType
    DR = mybir.MatmulPerfMode.DoubleRow

    nc = bacc.Bacc("TRN2", target_bir_lowering=False, debug=False, num_devices=NCORES)

    lg_in = nc.dram_tensor("logits", [SH, C], f32, kind="ExternalInput")
    em_in = nc.dram_tensor("emb", [SH, D], f32r, kind="ExternalInput")
    lab_in = nc.dram_tensor("labels_f", [128, T], f32, kind="ExternalInput")
    out_losses = nc.dram_tensor("partials", [128, 8], f32, kind="ExternalOutput")

    with tile.TileContext(nc) as tc:
        with (
            tc.tile_pool(name="const", bufs=1) as constp,
            tc.tile_pool(name="persist", bufs=1) as pers,
            tc.tile_pool(name="scratch", bufs=3) as scr,
            tc.tile_pool(name="scr1", bufs=1) as scr1,
            tc.tile_pool(name="scr2", bufs=2) as scr2,
            tc.tile_pool(name="psumB", bufs=3, space="PSUM") as pspB,
            tc.tile_pool(name="psumA", bufs=2, space="PSUM") as pspA,
            tc.tile_pool(name="dram", bufs=1, space="DRAM") as dram,
        ):
            _tiles = {}

            def PT(pool, shape, dtype, name):
                if name not in _tiles:
                    _tiles[name] = pool.tile(shape, dtype, name=name)
                return _tiles[name]

            def sim_ps(name):
                return pspB.tile([128, 1024], f32, name=name, tag="sim")

            def aux_ps(name):
                return pspA.tile([128, 512], f32, name=name, tag="aux")

            # ---------- constants (once) ----------
            ident = constp.tile([128, 128], f32, name="ident")
            make_identity(nc, ident)
            ident_r = constp.tile([128, 128], f32r, name="ident_r")
            nc.vector.tensor_copy(ident_r, ident)
            ones_c = constp.tile([128, 1], f32, name="ones_c")
            nc.vector.memset(ones_c, 1.0)
            ones2 = constp.tile([128, 2], f32, name="ones2")
            nc.vector.memset(ones2, 1.0)
            ones_r2 = constp.tile([128, 2], f32r, name="ones_r2")
            nc.vector.tensor_copy(ones_r2, ones2)
            onemI = constp.tile([128, 128], f32, name="onemI")
            nc.vector.memset(onemI, 1.0)
            nc.gpsimd.affine_select(
                out=onemI, in_=onemI, compare_op=OP.not_equal, fill=0.0,
                base=0, pattern=[[-1, 128]], channel_multiplier=1,
            )
            iota_i = constp.tile([128, C], i32, name="iota_i")
            nc.gpsimd.iota(iota_i, pattern=[[1, C]], base=0, channel_multiplier=0)
            iota_f = constp.tile([128, C], f32, name="iota_f")
            nc.vector.tensor_copy(iota_f, iota_i)

            lab = constp.tile([128, T], f32, name="lab")
            nc.sync.dma_start(lab, lab_in[:, :])

            pid = nc.sync.partition_id()

            for _it in range(unroll):
                # ---------- DRAM scratch ----------
                zt_local = dram.tile([D, SH], f8, name=f"zt_local{_it}")
                zt_gath = dram.tile(
                    [NCORES, D, SH], f8, name=f"zt_gath{_it}", addr_space="Shared"
                )
                seg_in = dram.tile([128, 4, 513], bf16, name=f"seg_in{_it}")
                seg_out = dram.tile(
                    [128, 4, 513], bf16, name=f"seg_out{_it}", addr_space="Shared"
                )

                # ---------- persistent tiles ----------
                # ez8[:, t, 0:256]=e, [256:512]=z, [512]=1.0
                ez8 = PT(pers, [128, T, 513], f32r, "ez8")
                O8 = PT(pers, [128, T, C], f32r, "O8")
                ztf2 = PT(pers, [128, 2, B], f8, f"ztf2_{_it % 2}")
                zts = [PT(pers, [128, SH], f8, f"zts{d}") for d in range(2)]
                ssqs = PT(pers, [128, T], f32, "ssqs")
                ce_sums = PT(pers, [128, T], f32, "ce_sums")
                gls = PT(pers, [128, T], f32, "gls")
                rsAll = PT(pers, [128, T, 8], f32, "rsAll")
                rowsums = PT(pers, [128, T], f32, "rowsums")
                zden = PT(pers, [128, T], f32, "zden")
                finals = PT(pers, [128, 8], f32, "finals")

                if _it == 0:
                    nc.vector.tensor_copy(
                        ez8[:, :, 512:513],
                        ones_c[:, 0:1, None].to_broadcast([128, T, 1]),
                    )

                # ================= Phase A =================
                nc.sync.dma_start(
                    ez8[:, :, 0:D],
                    em_in.rearrange("(t p) d -> p t d", p=128),
                )
                sq8 = scr1.tile([128, T, D], f32, name="sq8", tag="sq8")
                nc.vector.tensor_tensor(sq8, ez8[:, :, 0:D], ez8[:, :, 0:D], OP.mult)
                nc.vector.reduce_sum(ssqs, sq8, axis=AX.X)
                zl = PT(constp, [128, T], f32, "zl")
                nc.scalar.activation(zl, ssqs, ACT.Ln)
                nc.scalar.activation(zden, zl, ACT.Exp, scale=-0.5)

                # z = e * zden (broadcast); one-hot O8 = (iota == lab)
                nc.vector.tensor_tensor(
                    ez8[:, :, D:2 * D], ez8[:, :, 0:D],
                    zden[:, :, None].to_broadcast([128, T, D]), OP.mult,
                )
                nc.vector.tensor_tensor(
                    O8, iota_f[:, None, :].to_broadcast([128, T, C]),
                    lab[:, :, None].to_broadcast([128, T, C]), OP.is_equal,
                )

                # transpose z -> zts (one transpose per rotating aux tile)
                for d in range(2):
                    for t in range(T):
                        ptile = aux_ps(f"tr")
                        nc.tensor.transpose(
                            ptile[:, 0:128].bitcast(f32r),
                            ez8[:, t, D + d * 128:D + (d + 1) * 128],
                            ident_r,
                        )
                        nc.vector.tensor_scalar(
                            zts[d][:, t * 128:(t + 1) * 128], ptile[:, 0:128],
                            16.0, None, OP.mult,
                        )
                for d in range(2):
                    nc.sync.dma_start(zt_local[d * 128:(d + 1) * 128, :], zts[d])
                nc.gpsimd.collective_compute(
                    "AllGather", OP.bypass,
                    replica_groups=[list(range(NCORES))],
                    ins=[zt_local.opt()], outs=[zt_gath.opt()],
                )

                # segment matmuls: per class-block, main + count accumulators
                seg_sb = PT(pers, [128, 4, 513], bf16, "seg_sb")
                for cb in range(4):
                    smain = aux_ps("segm")
                    scnt = aux_ps("segc")
                    for t in range(T):
                        lhs = O8[:, t, cb * 128:(cb + 1) * 128]
                        nc.tensor.matmul(
                            smain[:, 0:512], lhs, ez8[:, t, 0:512],
                            start=(t == 0), stop=(t == T - 1),
                        )
                        nc.tensor.matmul(
                            scnt[:, 0:2], lhs, ones_r2,
                            start=(t == 0), stop=(t == T - 1),
                        )
                    nc.vector.tensor_copy(seg_sb[:, cb, 0:512], smain[:, 0:512])
                    nc.vector.tensor_copy(seg_sb[:, cb, 512:513], scnt[:, 0:1])
                nc.sync.dma_start(seg_in[:, :, :], seg_sb)
                nc.gpsimd.collective_compute(
                    "AllReduce", OP.add,
                    replica_groups=[list(range(NCORES))],
                    ins=[seg_in.opt()], outs=[seg_out.opt()],
                )

                # CE: one logits DMA; gather-own-logit batched; exp in-place per t
                lgt8 = scr2.tile([128, T, C], f32, name="lgt8", tag="lgt8")
                nc.sync.dma_start(
                    lgt8, lg_in.rearrange("(t p) d -> p t d", p=128)
                )
                gsc8 = scr1.tile([128, T, C], f32, name="gsc8", tag="gsc8")
                nc.vector.tensor_tensor(gsc8, O8, lgt8, OP.mult)
                nc.vector.reduce_sum(gls, gsc8, axis=AX.X)
                for t in range(T):
                    nc.scalar.activation(
                        lgt8[:, t, :], lgt8[:, t, :], ACT.Exp,
                        accum_out=ce_sums[:, t:t + 1],
                    )

                # gathered zT, rotated (own block first)
                for d in range(2):
                    nc.sync.dma_start(ztf2[:, d, 0:SH], zt_local[d * 128:(d + 1) * 128, :])
                for blk in range(1, NCORES):
                    src = (pid + blk) % NCORES
                    nc.sync.dma_start(
                        ztf2[:, :, blk * SH:(blk + 1) * SH],
                        zt_gath[bass.ds(src, 1), :, :].rearrange(
                            "x (d p) c -> p (x d) c", p=128),
                    )

                def proto_block():
                    # AllReduce-dependent class math; emitted mid-B so it
                    # overlaps the exp pipeline (AllReduce lands early in B).
                    sseg_h = PT(pers, [128, 4, 513], bf16, "sseg_h")
                    nc.sync.dma_start(sseg_h, seg_out[:, :, :])
                    sseg = PT(pers, [128, 4, 513], f32, "sseg")
                    nc.vector.tensor_copy(sseg, sseg_h)

                    cnts = PT(pers, [128, 4], f32, "cnts")
                    nc.vector.tensor_copy(cnts[:, :, None], sseg[:, :, 512:513])
                    cntm = PT(pers, [128, 4], f32, "cntm")
                    nc.vector.tensor_scalar(cntm, cnts, 1.0, None, OP.max)
                    rcnt = PT(pers, [128, 4], f32, "rcnt")
                    nc.vector.reciprocal(rcnt, cntm)
                    cm1 = PT(pers, [128, 4], f32, "cm1")
                    nc.vector.tensor_scalar(cm1, cnts, -1.0, 1.0, OP.add, OP.max)
                    rcm1 = PT(pers, [128, 4], f32, "rcm1")
                    nc.vector.reciprocal(rcm1, cm1)
                    v2 = PT(pers, [128, 4], f32, "v2")
                    nc.vector.tensor_scalar(v2, cnts, 2.0, None, OP.is_ge)
                    v1 = PT(pers, [128, 4], f32, "v1")
                    nc.vector.tensor_scalar(v1, cnts, 0.5, None, OP.is_ge)

                    protos = PT(pers, [128, 4, D], f32, "protos")
                    nc.vector.tensor_tensor(
                        protos, sseg[:, :, 0:D],
                        rcnt[:, :, None].to_broadcast([128, 4, D]), OP.mult,
                    )
                    psq = scr1.tile([128, 4, D], f32, name="psq", tag="sq4")
                    nc.vector.tensor_tensor(psq, protos, protos, OP.mult)
                    pn2 = PT(pers, [128, 4], f32, "pn2")
                    nc.vector.reduce_sum(pn2, psq, axis=AX.X)
                    ssq2 = scr1.tile([128, 4, D], f32, name="ssq2", tag="sq4b")
                    nc.vector.tensor_tensor(
                        ssq2, sseg[:, :, D:2 * D], sseg[:, :, D:2 * D], OP.mult
                    )
                    S2 = PT(pers, [128, 4], f32, "S2")
                    nc.vector.reduce_sum(S2, ssq2, axis=AX.X)

                    t3 = PT(pers, [128, 4], f32, "t3")
                    nc.vector.tensor_tensor(t3, S2, cnts, OP.subtract)
                    nc.vector.tensor_scalar(t3, t3, INV_TAU, None, OP.mult)
                    nc.vector.tensor_tensor(t3, t3, rcm1, OP.mult)
                    nc.vector.tensor_tensor(t3, t3, v2, OP.mult)
                    nc.vector.reduce_sum(finals[:, 0:1], t3, axis=AX.X)
                    nval = scr.tile([128, 4], f32, name="nval", tag="s4")
                    nc.vector.tensor_tensor(nval, v2, cnts, OP.mult)
                    nc.vector.reduce_sum(finals[:, 2:3], nval, axis=AX.X)
                    cpn = scr.tile([128, 4], f32, name="cpn", tag="s4")
                    nc.vector.tensor_tensor(cpn, cnts, pn2, OP.mult)
                    nc.vector.reduce_sum(finals[:, 3:4], cpn, axis=AX.X)

                    pnm = scr.tile([128, 4], f32, name="pnm", tag="s4b")
                    nc.vector.tensor_scalar(pnm, pn2, 1e-30, None, OP.max)
                    pl = scr.tile([128, 4], f32, name="pl", tag="s4c")
                    nc.scalar.activation(pl, pnm, ACT.Ln)
                    pden = PT(pers, [128, 4], f32, "pden")
                    nc.scalar.activation(pden, pl, ACT.Exp, scale=-0.5)
                    nc.vector.tensor_tensor(pden, pden, v1, OP.mult)

                    pnz = PT(pers, [128, 4, D], f32r, "pnz")
                    nc.vector.tensor_tensor(
                        pnz, protos, pden[:, :, None].to_broadcast([128, 4, D]),
                        OP.mult,
                    )
                    dsq = scr1.tile([128, 4, D], f32, name="dsq", tag="sq4")
                    nc.vector.tensor_tensor(dsq, pnz, pnz, OP.mult)
                    d2 = PT(pers, [128, 4], f32, "d2")
                    nc.vector.reduce_sum(d2, dsq, axis=AX.X)

                    pnzT = [PT(pers, [128, C], f32r, f"pnzT{d}") for d in range(2)]
                    for cb in range(4):
                        for d in range(2):
                            gt = aux_ps("gt")
                            nc.tensor.transpose(
                                gt[:, 0:128].bitcast(f32r),
                                pnz[:, cb, d * 128:(d + 1) * 128],
                                ident_r,
                            )
                            nc.vector.tensor_copy(
                                pnzT[d][:, cb * 128:(cb + 1) * 128], gt[:, 0:128]
                            )
                    g2 = PT(pers, [128, 4], f32, "g2")
                    for cb in range(4):
                        gp = aux_ps("gp")
                        for d in range(2):
                            nc.tensor.matmul(
                                gp[:, :],
                                pnzT[d][:, cb * 128:(cb + 1) * 128],
                                pnzT[d][:, :],
                                start=(d == 0), stop=(d == 1),
                            )
                        nc.scalar.activation(
                            gp[:, :], gp[:, :], ACT.Square,
                            accum_out=g2[:, cb:cb + 1],
                        )
                    d2sq = scr.tile([128, 4], f32, name="d2sq", tag="s4")
                    nc.vector.tensor_tensor(d2sq, d2, d2, OP.mult)
                    g2r = scr.tile([128, 1], f32, name="g2r", tag="rst")
                    nc.vector.reduce_sum(g2r, g2, axis=AX.X)
                    d2r = scr.tile([128, 1], f32, name="d2r", tag="rst")
                    nc.vector.reduce_sum(d2r, d2sq, axis=AX.X)
                    nc.vector.tensor_tensor(finals[:, 4:5], g2r, d2r, OP.subtract)
                    nc.vector.reduce_sum(finals[:, 5:6], v1, axis=AX.X)

                # ================= Phase B =================
                # [128,1024] sim tiles; 8 col-chunks per row-tile; rsAll [128,T,8]
                for r in range(T):
                    lhsT = ztf2[:, :, r * 128:(r + 1) * 128]
                    for jc in range(8):
                        ps = sim_ps("sim")
                        for jb in range(2):
                            c0 = jc * 1024 + jb * 512
                            nc.tensor.matmul(
                                ps[:, jb * 512:(jb + 1) * 512],
                                lhsT,
                                ztf2[:, :, c0:c0 + 512],
                                start=True, stop=True, perf_mode=DR,
                            )
                        if jc == 0:
                            nc.vector.tensor_tensor(
                                ps[:, r * 128:(r + 1) * 128],
                                ps[:, r * 128:(r + 1) * 128], onemI, OP.mult,
                            )
                        if jc >= 8 - DVE_NJC:
                            exd = scr.tile([128, 1024], i16, name="exd", tag="exd")
                            nc.vector.tensor_scalar(
                                exd, ps, SCH_A, SCH_B, OP.mult, OP.add
                            )
                            nc.vector.reduce_sum(
                                rsAll[:, r, jc:jc + 1], exd.bitcast(bf16), axis=AX.X
                            )
                        else:
                            nc.scalar.activation(
                                ps, ps, ACT.Exp, scale=INV_TAU / 256.0,
                                accum_out=rsAll[:, r, jc:jc + 1],
                            )
                    if r == 3:
                        proto_block()
                # all row sums at once; remove the exp(0)=1 from the zeroed diag
                nc.vector.reduce_sum(rowsums, rsAll, axis=AX.X)
                nc.vector.tensor_scalar(rowsums, rowsums, -1.0, None, OP.add)

                # ================= Phase C tail (rowsum-dependent) =============
                lse = PT(pers, [128, T], f32r, "lse")
                nc.scalar.activation(lse, rowsums, ACT.Ln)
                lse_ce = PT(pers, [128, T], f32, "lse_ce")
                nc.scalar.activation(lse_ce, ce_sums, ACT.Ln)

                ced = scr.tile([128, T], f32, name="ced", tag="ced")
                nc.vector.tensor_tensor(ced, lse_ce, gls, OP.subtract)
                celoc = PT(pers, [128, 1], f32, "celoc")
                nc.vector.reduce_sum(celoc, ced, axis=AX.X)
                sseloc = PT(pers, [128, 1], f32, "sseloc")
                nc.vector.reduce_sum(sseloc, ssqs, axis=AX.X)

                lse2 = PT(pers, [128, T, 2], f32r, "lse2")
                if _it == 0:
                    nc.vector.tensor_copy(
                        lse2[:, :, 1:2],
                        ones_c[:, 0:1, None].to_broadcast([128, T, 1]),
                    )
                nc.vector.tensor_copy(lse2[:, :, 0:1], lse[:, :, None])
                lsS = PT(pers, [128, 4], f32, "lsS")
                v2p = PT(pers, [128, 4], f32, "v2")
                for pair in range(2):
                    ltiles = [aux_ps("lse") for _ in range(2)]
                    for t in range(T):
                        for k in range(2):
                            cb = pair * 2 + k
                            nc.tensor.matmul(
                                ltiles[k][:, 0:2],
                                O8[:, t, cb * 128:(cb + 1) * 128],
                                lse2[:, t, :],
                                start=(t == 0), stop=(t == T - 1),
                            )
                    for k in range(2):
                        nc.vector.tensor_copy(
                            lsS[:, pair * 2 + k:pair * 2 + k + 1], ltiles[k][:, 0:1]
                        )
                nc.vector.tensor_tensor(lsS, lsS, v2p, OP.mult)
                nc.vector.reduce_sum(finals[:, 1:2], lsS, axis=AX.X)

                nc.vector.tensor_copy(finals[:, 6:7], celoc)
                nc.vector.tensor_copy(finals[:, 7:8], sseloc)

                nc.sync.dma_start(out_losses[:, :], finals)

    nc.compile()
    return nc


def _get_nc():
    if "nc" not in _CACHE:
        _CACHE["nc"] = _build()
    return _CACHE["nc"]


def kernel(logits, embeddings, labels):
    from concourse import bass_utils

    nc = _get_nc()

    logits = np.ascontiguousarray(np.asarray(logits, dtype=np.float32))
    embeddings = np.ascontiguousarray(np.asarray(embeddings, dtype=np.float32))
    labels_np = np.asarray(labels)

    in_maps = []
    for c in range(NCORES):
        sl = slice(c * SH, (c + 1) * SH)
        lab_f = labels_np[sl].astype(np.float32).reshape(T, 128).T
        in_maps.append({
            "logits": logits[sl],
            "emb": embeddings[sl],
            "labels_f": np.ascontiguousarray(lab_f),
        })

    res = bass_utils.run_bass_kernel_spmd(nc, in_maps, core_ids=list(range(NCORES)))

    p0 = res.results[0]["partials"].astype(np.float64)
    t3a = p0[:, 0].sum()
    nvalid = p0[:, 2].sum()
    cntpn2 = p0[:, 3].sum()
    l4num = p0[:, 4].sum()
    npres = p0[:, 5].sum()
    t3b = ce = sse = 0.0
    for c in range(NCORES):
        pc = res.results[c]["partials"].astype(np.float64)
        t3b += pc[:, 1].sum()
        ce += pc[:, 6].sum()
        sse += pc[:, 7].sum()

    l1 = ce / B
    l2 = (sse - cntpn2) / B
    l3 = -(t3a - t3b) / max(nvalid, 1.0)
    l4 = l4num / max(npres * npres - npres, 1.0)
    total = l1 + ALPHA * l2 + BETA * l3 + GAMMA * l4
    return tuple(np.float32(v) for v in (total, l1, l2, l3, l4))


# revision 7
# speedup vs baseline: 1.2392x; 1.2392x over previous
# Trainium2 Bass kernel for nn_CombinedLoss — v6
#
# v6 = v4 (single rotating PSUM pool of 2x[128,2048], fp8 DoubleRow sim
# matmuls, ACT/DVE exp split, batched prep ops) + SOFTWARE-PIPELINED EMISSION:
# the unrolled loop emits phase A of iteration k+1 BEFORE phase B/C of
# iteration k, with parity-double-buffered ez8/O8/ztf2 (+ small per-iteration
# scalars). Each engine's instruction stream then flows without stalling on
# the transpose->AllGather->load backbone: prep(k+1) work fills the gaps
# while iteration k's sim/exp pipeline runs.
#
# Output: partials per core, reduced on host exactly like the baseline.

import numpy as np

B = 8192
C = 512
D = 256
NCORES = 8
SH = B // NCORES
T = SH // 128
ALPHA = 0.5
BETA = 0.5
GAMMA = 0.5
INV_TAU = 10.0
EPS = 1e-8
UNROLL = 128

SCH_A = 184.6649652337873 * (INV_TAU / 256.0)
SCH_B = 16248.78071298956

# sim (r, jc) 2048-col blocks on DVE (Schraudolph); jc==0 holds the diagonal.
DVE_BLOCKS = {(r, 3) for r in range(T)}

_CACHE = {}


def _build(unroll=UNROLL):
    import concourse.bass as bass
    import concourse.mybir as mybir
    import concourse.tile as tile
    from concourse import bacc
    from concourse.masks import make_identity

    f32 = mybir.dt.float32
    f32r = mybir.dt.float32r
    bf16 = mybir.dt.bfloat16
    f8 = mybir.dt.float8e4
    i16 = mybir.dt.int16
    i32 = mybir.dt.int32
    AX = mybir.AxisListType
    OP = mybir.AluOpType
    ACT = mybir.ActivationFunctionType
    DR = mybir.MatmulPerfMode.DoubleRow

    nc = bacc.Bacc("TRN2", target_bir_lowering=False, debug=False, num_devices=NCORES)

    lg_in = nc.dram_tensor("logits", [SH, C], f32, kind="ExternalInput")
    em_in = nc.dram_tensor("emb", [SH, D], f32r, kind="ExternalInput")
    lab_in = nc.dram_tensor("labels_f", [128, T], f32, kind="ExternalInput")
    out_losses = nc.dram_tensor("partials", [128, 8], f32, kind="ExternalOutput")

    with tile.TileContext(nc) as tc:
        with (
            tc.tile_pool(name="const", bufs=1) as constp,
            tc.tile_pool(name="persist", bufs=1) as pers,
            tc.tile_pool(name="scratch", bufs=3) as scr,
            tc.tile_pool(name="scr1", bufs=1) as scr1,
            tc.tile_pool(name="scr2", bufs=2) as scr2,
            tc.tile_pool(name="psum8", bufs=2, space="PSUM") as psp,
            tc.tile_pool(name="dram", bufs=1, space="DRAM") as dram,
        ):
            _tiles = {}

            def PT(pool, shape, dtype, name):
                if name not in _tiles:
                    _tiles[name] = pool.tile(shape, dtype, name=name)
                return _tiles[name]

            def big_ps(name):
                return psp.tile([128, 2048], f32, name=name, tag="big")

            # ---------- constants ----------
            ident = constp.tile([128, 128], f32, name="ident")
            make_identity(nc, ident)
            ident_r = constp.tile([128, 128], f32r, name="ident_r")
            nc.vector.tensor_copy(ident_r, ident)
            ones_c = constp.tile([128, 1], f32, name="ones_c")
            nc.vector.memset(ones_c, 1.0)
            ones2 = constp.tile([128, 2], f32, name="ones2")
            nc.vector.memset(ones2, 1.0)
            ones_r2 = constp.tile([128, 2], f32r, name="ones_r2")
            nc.vector.tensor_copy(ones_r2, ones2)
            onemI = constp.tile([128, 128], f32, name="onemI")
            nc.vector.memset(onemI, 1.0)
            nc.gpsimd.affine_select(
                out=onemI, in_=onemI, compare_op=OP.not_equal, fill=0.0,
                base=0, pattern=[[-1, 128]], channel_multiplier=1,
            )
            iota_i = constp.tile([128, C], i32, name="iota_i")
            nc.gpsimd.iota(iota_i, pattern=[[1, C]], base=0, channel_multiplier=0)
            iota_f = constp.tile([128, C], f32, name="iota_f")
            nc.vector.tensor_copy(iota_f, iota_i)

            lab = constp.tile([128, T], f32, name="lab")
            nc.sync.dma_start(lab, lab_in[:, :])

            pid = nc.sync.partition_id()

            # parity-indexed persistent buffers (iteration j uses p = j % 2)
            def bufs(p):
                return dict(
                    ez8=PT(pers, [128, T, 513], f32r, f"ez8_{p}"),
                    O8=PT(pers, [128, T, C], f32r, f"O8_{p}"),
                    ztf2=PT(pers, [128, 2, B], f8, f"ztf2_{p}"),
                    ssqs=PT(pers, [128, T], f32, f"ssqs_{p}"),
                    zden=PT(pers, [128, T], f32, f"zden_{p}"),
                    ce_sums=PT(pers, [128, T], f32, f"ce_sums_{p}"),
                    gls=PT(pers, [128, T], f32, f"gls_{p}"),
                )

            zts = [PT(pers, [128, SH], f8, f"zts{d}") for d in range(2)]
            seg_sb = PT(pers, [128, 4, 513], bf16, "seg_sb")
            rsAll = PT(pers, [128, T, 4], f32, "rsAll")
            rowsums = PT(pers, [128, T], f32, "rowsums")
            finals = PT(pers, [128, 8], f32, "finals")

            dramb = {}

            def prep_head(j):
                """DRAM scratch + embeddings DMA + row sumsq for iteration j."""
                p = j % 2
                bb = bufs(p)
                ez8, ssqs = bb["ez8"], bb["ssqs"]

                zt_local = dram.tile([D, SH], f8, name=f"zt_local{j}")
                zt_gath = dram.tile(
                    [NCORES, D, SH], f8, name=f"zt_gath{j}", addr_space="Shared"
                )
                seg_in = dram.tile([128, 4, 513], bf16, name=f"seg_in{j}")
                seg_out = dram.tile(
                    [128, 4, 513], bf16, name=f"seg_out{j}", addr_space="Shared"
                )
                dramb[j] = (zt_local, zt_gath, seg_in, seg_out)

                if j < 2:
                    nc.vector.tensor_copy(
                        ez8[:, :, 512:513],
                        ones_c[:, 0:1, None].to_broadcast([128, T, 1]),
                    )
                nc.sync.dma_start(
                    ez8[:, :, 0:D], em_in.rearrange("(t p) d -> p t d", p=128)
                )
                sq8 = scr1.tile([128, T, D], f32, name="sq8", tag="sq8")
                nc.vector.tensor_tensor(sq8, ez8[:, :, 0:D], ez8[:, :, 0:D], OP.mult)
                nc.vector.reduce_sum(ssqs, sq8, axis=AX.X)

            def prep_znorm(j):
                p = j % 2
                bb = bufs(p)
                ez8, O8, ssqs, zden = bb["ez8"], bb["O8"], bb["ssqs"], bb["zden"]
                zl = scr.tile([128, T], f32, name="zl", tag="zl")
                nc.scalar.activation(zl, ssqs, ACT.Ln)
                nc.scalar.activation(zden, zl, ACT.Exp, scale=-0.5)
                nc.vector.tensor_tensor(
                    ez8[:, :, D:2 * D], ez8[:, :, 0:D],
                    zden[:, :, None].to_broadcast([128, T, D]), OP.mult,
                )
                nc.vector.tensor_tensor(
                    O8, iota_f[:, None, :].to_broadcast([128, T, C]),
                    lab[:, :, None].to_broadcast([128, T, C]), OP.is_equal,
                )

            def prep_gather(j):
                p = j % 2
                ez8 = bufs(p)["ez8"]
                zt_local, zt_gath, _, _ = dramb[j]
                for d in range(2):
                    for g in range(2):
                        ptile = big_ps(f"tr{d}{g}")
                        for k in range(4):
                            t = g * 4 + k
                            nc.tensor.transpose(
                                ptile[:, k * 512:k * 512 + 128].bitcast(f32r),
                                ez8[:, t, D + d * 128:D + (d + 1) * 128],
                                ident_r,
                            )
                        src = ptile.rearrange("p (k c) -> p k c", k=4)[:, :, 0:128]
                        dst = zts[d].rearrange("p (g k c) -> p g k c", g=2, k=4)[:, g]
                        nc.vector.tensor_scalar(dst, src, 16.0, None, OP.mult)
                for d in range(2):
                    nc.sync.dma_start(zt_local[d * 128:(d + 1) * 128, :], zts[d])
                nc.gpsimd.collective_compute(
                    "AllGather", OP.bypass,
                    replica_groups=[list(range(NCORES))],
                    ins=[zt_local.opt()], outs=[zt_gath.opt()],
                )

            def prep_seg(j):
                p = j % 2
                bb = bufs(p)
                ez8, O8 = bb["ez8"], bb["O8"]
                _, _, seg_in, seg_out = dramb[j]
                for half in range(2):
                    stile = big_ps(f"seg{half}")
                    for t in range(T):
                        for k in range(2):
                            cb = half * 2 + k
                            lhs = O8[:, t, cb * 128:(cb + 1) * 128]
                            nc.tensor.matmul(
                                stile[:, k * 1024:k * 1024 + 512], lhs,
                                ez8[:, t, 0:512],
                                start=(t == 0), stop=(t == T - 1),
                            )
                            nc.tensor.matmul(
                                stile[:, k * 1024 + 512:k * 1024 + 514], lhs, ones_r2,
                                start=(t == 0), stop=(t == T - 1),
                            )
                    nc.vector.tensor_copy(
                        seg_sb[:, half * 2:half * 2 + 2, :],
                        stile.rearrange("p (k c) -> p k c", k=2)[:, :, 0:513],
                    )
                nc.sync.dma_start(seg_in[:, :, :], seg_sb)
                nc.gpsimd.collective_compute(
                    "AllReduce", OP.add,
                    replica_groups=[list(range(NCORES))],
                    ins=[seg_in.opt()], outs=[seg_out.opt()],
                )

            def prep_ce(j):
                p = j % 2
                bb = bufs(p)
                O8, ce_sums, gls = bb["O8"], bb["ce_sums"], bb["gls"]
                lgt8 = scr1.tile([128, T, C], f32, name="lgt8", tag="lgt8")
                nc.sync.dma_start(lgt8, lg_in.rearrange("(t p) d -> p t d", p=128))
                gsc8 = scr1.tile([128, T, C], bf16, name="gsc8", tag="gsc8")
                nc.vector.tensor_tensor(gsc8, O8, lgt8, OP.mult)
                nc.vector.reduce_sum(gls, gsc8, axis=AX.X)
                for t in range(T):
                    nc.scalar.activation(
                        lgt8[:, t, :], lgt8[:, t, :], ACT.Exp,
                        accum_out=ce_sums[:, t:t + 1],
                    )

            def prep_loads(j):
                """ztf2 loads for iteration j (waits on its AllGather)."""
                p = j % 2
                ztf2 = bufs(p)["ztf2"]
                zt_local, zt_gath, _, _ = dramb[j]
                for d in range(2):
                    nc.sync.dma_start(
                        ztf2[:, d, 0:SH], zt_local[d * 128:(d + 1) * 128, :]
                    )
                for blk in range(1, NCORES):
                    src = (pid + blk) % NCORES
                    nc.sync.dma_start(
                        ztf2[:, :, blk * SH:(blk + 1) * SH],
                        zt_gath[bass.ds(src, 1), :, :].rearrange(
                            "x (d p) c -> p (x d) c", p=128),
                    )

            def proto_block(j):
                """AllReduce-dependent class math (emitted mid-B)."""
                _, _, _, seg_out = dramb[j]
                sseg_h = PT(pers, [128, 4, 513], bf16, "sseg_h")
                nc.sync.dma_start(sseg_h, seg_out[:, :, :])
                sseg = PT(pers, [128, 4, 513], f32, "sseg")
                nc.vector.tensor_copy(sseg, sseg_h)

                cnts = PT(pers, [128, 4], f32, "cnts")
                nc.vector.tensor_copy(cnts[:, :, None], sseg[:, :, 512:513])
                cntm = PT(pers, [128, 4], f32, "cntm")
                nc.vector.tensor_scalar(cntm, cnts, 1.0, None, OP.max)
                rcnt = PT(pers, [128, 4], f32, "rcnt")
                nc.vector.reciprocal(rcnt, cntm)
                cm1 = PT(pers, [128, 4], f32, "cm1")
                nc.vector.tensor_scalar(cm1, cnts, -1.0, 1.0, OP.add, OP.max)
                rcm1 = PT(pers, [128, 4], f32, "rcm1")
                nc.vector.reciprocal(rcm1, cm1)
                v2 = PT(pers, [128, 4], f32, "v2")
                nc.vector.tensor_scalar(v2, cnts, 2.0, None, OP.is_ge)
                v1 = PT(pers, [128, 4], f32, "v1")
                nc.vector.tensor_scalar(v1, cnts, 0.5, None, OP.is_ge)

                protos = PT(pers, [128, 4, D], f32, "protos")
                nc.vector.tensor_tensor(
                    protos, sseg[:, :, 0:D],
                    rcnt[:, :, None].to_broadcast([128, 4, D]), OP.mult,
                )
                psq = scr1.tile([128, 4, D], f32, name="psq", tag="sq4")
                nc.vector.tensor_tensor(psq, protos, protos, OP.mult)
                pn2 = PT(pers, [128, 4], f32, "pn2")
                nc.vector.reduce_sum(pn2, psq, axis=AX.X)
                ssq2 = scr1.tile([128, 4, D], f32, name="ssq2", tag="sq4b")
                nc.vector.tensor_tensor(
                    ssq2, sseg[:, :, D:2 * D], sseg[:, :, D:2 * D], OP.mult
                )
                S2 = PT(pers, [128, 4], f32, "S2")
                nc.vector.reduce_sum(S2, ssq2, axis=AX.X)

                t3 = PT(pers, [128, 4], f32, "t3")
                nc.vector.tensor_tensor(t3, S2, cnts, OP.subtract)
                nc.vector.tensor_scalar(t3, t3, INV_TAU, None, OP.mult)
                nc.vector.tensor_tensor(t3, t3, rcm1, OP.mult)
                nc.vector.tensor_tensor(t3, t3, v2, OP.mult)
                nc.vector.reduce_sum(finals[:, 0:1], t3, axis=AX.X)
                nval = scr.tile([128, 4], f32, name="nval", tag="s4")
                nc.vector.tensor_tensor(nval, v2, cnts, OP.mult)
                nc.vector.reduce_sum(finals[:, 2:3], nval, axis=AX.X)
                cpn = scr.tile([128, 4], f32, name="cpn", tag="s4")
                nc.vector.tensor_tensor(cpn, cnts, pn2, OP.mult)
                nc.vector.reduce_sum(finals[:, 3:4], cpn, axis=AX.X)

                pnm = scr.tile([128, 4], f32, name="pnm", tag="s4b")
                nc.vector.tensor_scalar(pnm, pn2, 1e-30, None, OP.max)
                pl = scr.tile([128, 4], f32, name="pl", tag="s4c")
                nc.scalar.activation(pl, pnm, ACT.Ln)
                pden = PT(pers, [128, 4], f32, "pden")
                nc.scalar.activation(pden, pl, ACT.Exp, scale=-0.5)
                nc.vector.tensor_tensor(pden, pden, v1, OP.mult)

                pnz = PT(pers, [128, 4, D], f32r, "pnz")
                nc.vector.tensor_tensor(
                    pnz, protos, pden[:, :, None].to_broadcast([128, 4, D]), OP.mult
                )
                dsq = scr1.tile([128, 4, D], f32, name="dsq", tag="sq4")
                nc.vector.tensor_tensor(dsq, pnz, pnz, OP.mult)
                d2 = PT(pers, [128, 4], f32, "d2")
                nc.vector.reduce_sum(d2, dsq, axis=AX.X)

                pnzT = [PT(pers, [128, C], f32r, f"pnzT{d}") for d in range(2)]
                for g in range(2):
                    gt = big_ps(f"gt{g}")
                    for k in range(2):
                        cb = g * 2 + k
                        for d in range(2):
                            nc.tensor.transpose(
                                gt[:, (k * 2 + d) * 512:(k * 2 + d) * 512 + 128].bitcast(f32r),
                                pnz[:, cb, d * 128:(d + 1) * 128],
                                ident_r,
                            )
                    for d in range(2):
                        src = gt.rearrange("p (k x c) -> p k x c", k=2, x=2)[
                            :, :, d, 0:128]
                        dst = pnzT[d].rearrange("p (g k c) -> p g k c", g=2, k=2)[:, g]
                        nc.vector.tensor_copy(dst, src)

                g2 = PT(pers, [128, 4], f32, "g2")
                gp = big_ps("gp")
                for cb in range(4):
                    for d in range(2):
                        nc.tensor.matmul(
                            gp[:, cb * 512:(cb + 1) * 512],
                            pnzT[d][:, cb * 128:(cb + 1) * 128],
                            pnzT[d][:, :],
                            start=(d == 0), stop=(d == 1),
                        )
                for cb in range(4):
                    nc.scalar.activation(
                        gp[:, cb * 512:(cb + 1) * 512],
                        gp[:, cb * 512:(cb + 1) * 512],
                        ACT.Square, accum_out=g2[:, cb:cb + 1],
                    )
                d2sq = scr.tile([128, 4], f32, name="d2sq", tag="s4")
                nc.vector.tensor_tensor(d2sq, d2, d2, OP.mult)
                g2r = scr.tile([128, 1], f32, name="g2r", tag="rst")
                nc.vector.reduce_sum(g2r, g2, axis=AX.X)
                d2r = scr.tile([128, 1], f32, name="d2r", tag="rst")
                nc.vector.reduce_sum(d2r, d2sq, axis=AX.X)
                nc.vector.tensor_tensor(finals[:, 4:5], g2r, d2r, OP.subtract)
                nc.vector.reduce_sum(finals[:, 5:6], v1, axis=AX.X)

            def compute_rows(j, r0, r1):
                p = j % 2
                ztf2 = bufs(p)["ztf2"]
                for r in range(r0, r1):
                    lhsT = ztf2[:, :, r * 128:(r + 1) * 128]
                    for jc in range(4):
                        ps = big_ps("sim")
                        for jb in range(4):
                            c0 = jc * 2048 + jb * 512
                            nc.tensor.matmul(
                                ps[:, jb * 512:(jb + 1) * 512],
                                lhsT,
                                ztf2[:, :, c0:c0 + 512],
                                start=True, stop=True, perf_mode=DR,
                            )
                        if jc == 0:
                            nc.vector.tensor_tensor(
                                ps[:, r * 128:(r + 1) * 128],
                                ps[:, r * 128:(r + 1) * 128], onemI, OP.mult,
                            )
                        if (r, jc) in DVE_BLOCKS:
                            exd = scr2.tile([128, 2048], i16, name="exd", tag="exd")
                            nc.vector.tensor_scalar(
                                exd, ps, SCH_A, SCH_B, OP.mult, OP.add
                            )
                            nc.vector.reduce_sum(
                                rsAll[:, r, jc:jc + 1], exd.bitcast(bf16), axis=AX.X
                            )
                        else:
                            nc.scalar.activation(
                                ps, ps, ACT.Exp, scale=INV_TAU / 256.0,
                                accum_out=rsAll[:, r, jc:jc + 1],
                            )

            def tail(j):
                p = j % 2
                bb = bufs(p)
                O8, ssqs, ce_sums, gls = (
                    bb["O8"], bb["ssqs"], bb["ce_sums"], bb["gls"])
                nc.vector.reduce_sum(rowsums, rsAll, axis=AX.X)
                nc.vector.tensor_scalar(rowsums, rowsums, -1.0, None, OP.add)

                lse = PT(pers, [128, T], f32r, "lse")
                nc.scalar.activation(lse, rowsums, ACT.Ln)
                lse_ce = PT(pers, [128, T], f32, "lse_ce")
                nc.scalar.activation(lse_ce, ce_sums, ACT.Ln)

                ced = scr.tile([128, T], f32, name="ced", tag="ced")
                nc.vector.tensor_tensor(ced, lse_ce, gls, OP.subtract)
                celoc = PT(pers, [128, 1], f32, "celoc")
                nc.vector.reduce_sum(celoc, ced, axis=AX.X)
                sseloc = PT(pers, [128, 1], f32, "sseloc")
                nc.vector.reduce_sum(sseloc, ssqs, axis=AX.X)

                lse2 = PT(pers, [128, T, 2], f32r, "lse2")
                if j == 0:
                    nc.vector.tensor_copy(
                        lse2[:, :, 1:2],
                        ones_c[:, 0:1, None].to_broadcast([128, T, 1]),
                    )
                nc.vector.tensor_copy(lse2[:, :, 0:1], lse[:, :, None])
                lsetile = big_ps("lse")
                for t in range(T):
                    for cb in range(4):
                        nc.tensor.matmul(
                            lsetile[:, cb * 512:cb * 512 + 2],
                            O8[:, t, cb * 128:(cb + 1) * 128],
                            lse2[:, t, :],
                            start=(t == 0), stop=(t == T - 1),
                        )
                lsS = PT(pers, [128, 4], f32, "lsS")
                nc.vector.tensor_copy(
                    lsS[:, :, None],
                    lsetile.rearrange("p (c x) -> p c x", c=4)[:, :, 0:1],
                )
                v2p = PT(pers, [128, 4], f32, "v2")
                nc.vector.tensor_tensor(lsS, lsS, v2p, OP.mult)
                nc.vector.reduce_sum(finals[:, 1:2], lsS, axis=AX.X)

                nc.vector.tensor_copy(finals[:, 6:7], celoc)
                nc.vector.tensor_copy(finals[:, 7:8], sseloc)

                nc.sync.dma_start(out_losses[:, :], finals)

            # ---------- software-pipelined emission ----------
            prep_head(0)
            prep_znorm(0)
            prep_gather(0)
            prep_seg(0)
            prep_ce(0)
            prep_loads(0)
            for j in range(unroll):
                n = j + 1
                if n < unroll:
                    prep_head(n)
                compute_rows(j, 0, 2)
                if n < unroll:
                    prep_znorm(n)
                    prep_gather(n)
                compute_rows(j, 2, 4)
                if n < unroll:
                    prep_seg(n)
                    prep_ce(n)
                proto_block(j)
                compute_rows(j, 4, 8)
                tail(j)
                if n < unroll:
                    prep_loads(n)

    nc.compile()
    return nc


def _get_nc():
    if "nc" not in _CACHE:
        _CACHE["nc"] = _build()
    return _CACHE["nc"]


def kernel(logits, embeddings, labels):
    from concourse import bass_utils

    nc = _get_nc()

    logits = np.ascontiguousarray(np.asarray(logits, dtype=np.float32))
    embeddings = np.ascontiguousarray(np.asarray(embeddings, dtype=np.float32))
    labels_np = np.asarray(labels)

    in_maps = []
    for c in range(NCORES):
        sl = slice(c * SH, (c + 1) * SH)
        lab_f = labels_np[sl].astype(np.float32).reshape(T, 128).T
        in_maps.append({
            "logits": logits[sl],
            "emb": embeddings[sl],
            "labels_f": np.ascontiguousarray(lab_f),
        })

    res = bass_utils.run_bass_kernel_spmd(nc, in_maps, core_ids=list(range(NCORES)))

    p0 = res.results[0]["partials"].astype(np.float64)
    t3a = p0[:, 0].sum()
    nvalid = p0[:, 2].sum()
    cntpn2 = p0[:, 3].sum()
    l4num = p0[:, 4].sum()
    npres = p0[:, 5].sum()
    t3b = ce = sse = 0.0
    for c in range(NCORES):
        pc = res.results[c]["partials"].astype(np.float64)
        t3b += pc[:, 1].sum()
        ce += pc[:, 6].sum()
        sse += pc[:, 7].sum()

    l1 = ce / B
    l2 = (sse - cntpn2) / B
    l3 = -(t3a - t3b) / max(nvalid, 1.0)
    l4 = l4num / max(npres * npres - npres, 1.0)
    total = l1 + ALPHA * l2 + BETA * l3 + GAMMA * l4
    return tuple(np.float32(v) for v in (total, l1, l2, l3, l4))


# revision 8
# speedup vs baseline: 1.2564x; 1.0139x over previous
# Trainium2 Bass kernel for nn_CombinedLoss — v6
#
# v6 = v4 (single rotating PSUM pool of 2x[128,2048], fp8 DoubleRow sim
# matmuls, ACT/DVE exp split, batched prep ops) + SOFTWARE-PIPELINED EMISSION:
# the unrolled loop emits phase A of iteration k+1 BEFORE phase B/C of
# iteration k, with parity-double-buffered ez8/O8/ztf2 (+ small per-iteration
# scalars). Each engine's instruction stream then flows without stalling on
# the transpose->AllGather->load backbone: prep(k+1) work fills the gaps
# while iteration k's sim/exp pipeline runs.
#
# Output: partials per core, reduced on host exactly like the baseline.

import numpy as np

B = 8192
C = 512
D = 256
NCORES = 8
SH = B // NCORES
T = SH // 128
ALPHA = 0.5
BETA = 0.5
GAMMA = 0.5
INV_TAU = 10.0
EPS = 1e-8
UNROLL = 128

SCH_A = 184.6649652337873 * (INV_TAU / 256.0)
SCH_B = 16248.78071298956

# sim (r, jc) 2048-col blocks on DVE (Schraudolph); jc==0 holds the diagonal.
DVE_BLOCKS = {(r, 3) for r in range(6)}

_CACHE = {}


def _build(unroll=UNROLL):
    import concourse.bass as bass
    import concourse.mybir as mybir
    import concourse.tile as tile
    from concourse import bacc
    from concourse.masks import make_identity

    f32 = mybir.dt.float32
    f32r = mybir.dt.float32r
    bf16 = mybir.dt.bfloat16
    f8 = mybir.dt.float8e4
    i16 = mybir.dt.int16
    i32 = mybir.dt.int32
    AX = mybir.AxisListType
    OP = mybir.AluOpType
    ACT = mybir.ActivationFunctionType
    DR = mybir.MatmulPerfMode.DoubleRow

    nc = bacc.Bacc("TRN2", target_bir_lowering=False, debug=False, num_devices=NCORES)

    lg_in = nc.dram_tensor("logits", [SH, C], f32, kind="ExternalInput")
    em_in = nc.dram_tensor("emb", [SH, D], f32r, kind="ExternalInput")
    lab_in = nc.dram_tensor("labels_f", [128, T], f32, kind="ExternalInput")
    out_losses = nc.dram_tensor("partials", [128, 8], f32, kind="ExternalOutput")

    with tile.TileContext(nc) as tc:
        with (
            tc.tile_pool(name="const", bufs=1) as constp,
            tc.tile_pool(name="persist", bufs=1) as pers,
            tc.tile_pool(name="scratch", bufs=3) as scr,
            tc.tile_pool(name="scr1", bufs=1) as scr1,
            tc.tile_pool(name="scr2", bufs=2) as scr2,
            tc.tile_pool(name="psum8", bufs=2, space="PSUM") as psp,
            tc.tile_pool(name="dram", bufs=1, space="DRAM") as dram,
        ):
            _tiles = {}

            def PT(pool, shape, dtype, name):
                if name not in _tiles:
                    _tiles[name] = pool.tile(shape, dtype, name=name)
                return _tiles[name]

            def big_ps(name):
                return psp.tile([128, 2048], f32, name=name, tag="big")

            # ---------- constants ----------
            ident = constp.tile([128, 128], f32, name="ident")
            make_identity(nc, ident)
            ident_r = constp.tile([128, 128], f32r, name="ident_r")
            nc.vector.tensor_copy(ident_r, ident)
            ones_c = constp.tile([128, 1], f32, name="ones_c")
            nc.vector.memset(ones_c, 1.0)
            ones2 = constp.tile([128, 2], f32, name="ones2")
            nc.vector.memset(ones2, 1.0)
            ones_r2 = constp.tile([128, 2], f32r, name="ones_r2")
            nc.vector.tensor_copy(ones_r2, ones2)
            onemI = constp.tile([128, 128], f32, name="onemI")
            nc.vector.memset(onemI, 1.0)
            nc.gpsimd.affine_select(
                out=onemI, in_=onemI, compare_op=OP.not_equal, fill=0.0,
                base=0, pattern=[[-1, 128]], channel_multiplier=1,
            )
            iota_i = constp.tile([128, C], i32, name="iota_i")
            nc.gpsimd.iota(iota_i, pattern=[[1, C]], base=0, channel_multiplier=0)
            iota_f = constp.tile([128, C], f32, name="iota_f")
            nc.vector.tensor_copy(iota_f, iota_i)

            lab = constp.tile([128, T], f32, name="lab")
            nc.sync.dma_start(lab, lab_in[:, :])

            pid = nc.sync.partition_id()

            # parity-indexed persistent buffers (iteration j uses p = j % 2)
            def bufs(p):
                return dict(
                    ez8=PT(pers, [128, T, 513], f32r, f"ez8_{p}"),
                    O8=PT(pers, [128, T, C], f32r, f"O8_{p}"),
                    ztf2=PT(pers, [128, 2, B], f8, f"ztf2_{p}"),
                    ssqs=PT(pers, [128, T], f32, f"ssqs_{p}"),
                    zden=PT(pers, [128, T], f32, f"zden_{p}"),
                    ce_sums=PT(pers, [128, T], f32, f"ce_sums_{p}"),
                    gls=PT(pers, [128, T], f32, f"gls_{p}"),
                )

            zts = [PT(pers, [128, SH], f8, f"zts{d}") for d in range(2)]
            seg_sb = PT(pers, [128, 4, 513], bf16, "seg_sb")
            rsAll = PT(pers, [128, T, 4], f32, "rsAll")
            rowsums = PT(pers, [128, T], f32, "rowsums")
            finals = PT(pers, [128, 8], f32, "finals")

            dramb = {}

            def prep_head(j):
                """DRAM scratch + embeddings DMA + row sumsq for iteration j."""
                p = j % 2
                bb = bufs(p)
                ez8, ssqs = bb["ez8"], bb["ssqs"]

                zt_local = dram.tile([D, SH], f8, name=f"zt_local{j}")
                zt_gath = dram.tile(
                    [NCORES, D, SH], f8, name=f"zt_gath{j}", addr_space="Shared"
                )
                seg_in = dram.tile([128, 4, 513], bf16, name=f"seg_in{j}")
                seg_out = dram.tile(
                    [128, 4, 513], bf16, name=f"seg_out{j}", addr_space="Shared"
                )
                dramb[j] = (zt_local, zt_gath, seg_in, seg_out)

                if j < 2:
                    nc.vector.tensor_copy(
                        ez8[:, :, 512:513],
                        ones_c[:, 0:1, None].to_broadcast([128, T, 1]),
                    )
                nc.sync.dma_start(
                    ez8[:, :, 0:D], em_in.rearrange("(t p) d -> p t d", p=128)
                )
                for t in range(T):
                    sq = scr.tile([128, D], f32, name="sq", tag="sq")
                    nc.vector.scalar_tensor_tensor(
                        out=sq, in0=ez8[:, t, 0:D], scalar=1.0, in1=ez8[:, t, 0:D],
                        op0=OP.mult, op1=OP.mult, accum_out=ssqs[:, t:t + 1],
                    )

            def prep_znorm(j):
                p = j % 2
                bb = bufs(p)
                ez8, O8, ssqs, zden = bb["ez8"], bb["O8"], bb["ssqs"], bb["zden"]
                zl = scr.tile([128, T], f32, name="zl", tag="zl")
                nc.scalar.activation(zl, ssqs, ACT.Ln)
                nc.scalar.activation(zden, zl, ACT.Exp, scale=-0.5)
                nc.vector.tensor_tensor(
                    ez8[:, :, D:2 * D], ez8[:, :, 0:D],
                    zden[:, :, None].to_broadcast([128, T, D]), OP.mult,
                )
                for t in range(T):
                    nc.vector.tensor_scalar(
                        O8[:, t, :], iota_f, lab[:, t:t + 1], None, OP.is_equal
                    )

            def prep_gather(j):
                p = j % 2
                ez8 = bufs(p)["ez8"]
                zt_local, zt_gath, _, _ = dramb[j]
                for d in range(2):
                    for g in range(2):
                        ptile = big_ps(f"tr{d}{g}")
                        for k in range(4):
                            t = g * 4 + k
                            nc.tensor.transpose(
                                ptile[:, k * 512:k * 512 + 128].bitcast(f32r),
                                ez8[:, t, D + d * 128:D + (d + 1) * 128],
                                ident_r,
                            )
                        src = ptile.rearrange("p (k c) -> p k c", k=4)[:, :, 0:128]
                        dst = zts[d].rearrange("p (g k c) -> p g k c", g=2, k=4)[:, g]
                        nc.vector.tensor_scalar(dst, src, 16.0, None, OP.mult)
                for d in range(2):
                    nc.sync.dma_start(zt_local[d * 128:(d + 1) * 128, :], zts[d])
                nc.gpsimd.collective_compute(
                    "AllGather", OP.bypass,
                    replica_groups=[list(range(NCORES))],
                    ins=[zt_local.opt()], outs=[zt_gath.opt()],
                )

            def prep_seg(j):
                p = j % 2
                bb = bufs(p)
                ez8, O8 = bb["ez8"], bb["O8"]
                _, _, seg_in, seg_out = dramb[j]
                for half in range(2):
                    stile = big_ps(f"seg{half}")
                    for t in range(T):
                        for k in range(2):
                            cb = half * 2 + k
                            lhs = O8[:, t, cb * 128:(cb + 1) * 128]
                            nc.tensor.matmul(
                                stile[:, k * 1024:k * 1024 + 512], lhs,
                                ez8[:, t, 0:512],
                                start=(t == 0), stop=(t == T - 1),
                            )
                            nc.tensor.matmul(
                                stile[:, k * 1024 + 512:k * 1024 + 514], lhs, ones_r2,
                                start=(t == 0), stop=(t == T - 1),
                            )
                    nc.vector.tensor_copy(
                        seg_sb[:, half * 2:half * 2 + 2, :],
                        stile.rearrange("p (k c) -> p k c", k=2)[:, :, 0:513],
                    )
                nc.sync.dma_start(seg_in[:, :, :], seg_sb)
                nc.gpsimd.collective_compute(
                    "AllReduce", OP.add,
                    replica_groups=[list(range(NCORES))],
                    ins=[seg_in.opt()], outs=[seg_out.opt()],
                )

            def prep_ce(j):
                p = j % 2
                bb = bufs(p)
                O8, ce_sums, gls = bb["O8"], bb["ce_sums"], bb["gls"]
                lgt8 = scr1.tile([128, T, C], f32, name="lgt8", tag="lgt8")
                nc.sync.dma_start(lgt8, lg_in.rearrange("(t p) d -> p t d", p=128))
                for t in range(T):
                    gsc = scr.tile([128, C], f32, name="gsc", tag="gsc")
                    nc.vector.scalar_tensor_tensor(
                        out=gsc, in0=O8[:, t, :], scalar=1.0, in1=lgt8[:, t, :],
                        op0=OP.mult, op1=OP.mult, accum_out=gls[:, t:t + 1],
                    )
                for t in range(T):
                    nc.scalar.activation(
                        lgt8[:, t, :], lgt8[:, t, :], ACT.Exp,
                        accum_out=ce_sums[:, t:t + 1],
                    )

            def prep_loads(j):
                """ztf2 loads for iteration j (waits on its AllGather)."""
                p = j % 2
                ztf2 = bufs(p)["ztf2"]
                zt_local, zt_gath, _, _ = dramb[j]
                for d in range(2):
                    nc.sync.dma_start(
                        ztf2[:, d, 0:SH], zt_local[d * 128:(d + 1) * 128, :]
                    )
                for blk in range(1, NCORES):
                    src = (pid + blk) % NCORES
                    nc.sync.dma_start(
                        ztf2[:, :, blk * SH:(blk + 1) * SH],
                        zt_gath[bass.ds(src, 1), :, :].rearrange(
                            "x (d p) c -> p (x d) c", p=128),
                    )

            def proto_block(j):
                """AllReduce-dependent class math (emitted mid-B)."""
                _, _, _, seg_out = dramb[j]
                sseg_h = PT(pers, [128, 4, 513], bf16, "sseg_h")
                nc.sync.dma_start(sseg_h, seg_out[:, :, :])
                sseg = PT(pers, [128, 4, 513], f32, "sseg")
                nc.vector.tensor_copy(sseg, sseg_h)

                cnts = PT(pers, [128, 4], f32, "cnts")
                nc.vector.tensor_copy(cnts[:, :, None], sseg[:, :, 512:513])
                cntm = PT(pers, [128, 4], f32, "cntm")
                nc.vector.tensor_scalar(cntm, cnts, 1.0, None, OP.max)
                rcnt = PT(pers, [128, 4], f32, "rcnt")
                nc.vector.reciprocal(rcnt, cntm)
                cm1 = PT(pers, [128, 4], f32, "cm1")
                nc.vector.tensor_scalar(cm1, cnts, -1.0, 1.0, OP.add, OP.max)
                rcm1 = PT(pers, [128, 4], f32, "rcm1")
                nc.vector.reciprocal(rcm1, cm1)
                v2 = PT(pers, [128, 4], f32, "v2")
                nc.vector.tensor_scalar(v2, cnts, 2.0, None, OP.is_ge)
                v1 = PT(pers, [128, 4], f32, "v1")
                nc.vector.tensor_scalar(v1, cnts, 0.5, None, OP.is_ge)

                protos = PT(pers, [128, 4, D], f32, "protos")
                nc.vector.tensor_tensor(
                    protos, sseg[:, :, 0:D],
                    rcnt[:, :, None].to_broadcast([128, 4, D]), OP.mult,
                )
                psq = scr1.tile([128, 4, D], f32, name="psq", tag="sq4")
                nc.vector.tensor_tensor(psq, protos, protos, OP.mult)
                pn2 = PT(pers, [128, 4], f32, "pn2")
                nc.vector.reduce_sum(pn2, psq, axis=AX.X)
                ssq2 = scr1.tile([128, 4, D], f32, name="ssq2", tag="sq4b")
                nc.vector.tensor_tensor(
                    ssq2, sseg[:, :, D:2 * D], sseg[:, :, D:2 * D], OP.mult
                )
                S2 = PT(pers, [128, 4], f32, "S2")
                nc.vector.reduce_sum(S2, ssq2, axis=AX.X)

                t3 = PT(pers, [128, 4], f32, "t3")
                nc.vector.tensor_tensor(t3, S2, cnts, OP.subtract)
                nc.vector.tensor_scalar(t3, t3, INV_TAU, None, OP.mult)
                nc.vector.tensor_tensor(t3, t3, rcm1, OP.mult)
                nc.vector.tensor_tensor(t3, t3, v2, OP.mult)
                nc.vector.reduce_sum(finals[:, 0:1], t3, axis=AX.X)
                nval = scr.tile([128, 4], f32, name="nval", tag="s4")
                nc.vector.tensor_tensor(nval, v2, cnts, OP.mult)
                nc.vector.reduce_sum(finals[:, 2:3], nval, axis=AX.X)
                cpn = scr.tile([128, 4], f32, name="cpn", tag="s4")
                nc.vector.tensor_tensor(cpn, cnts, pn2, OP.mult)
                nc.vector.reduce_sum(finals[:, 3:4], cpn, axis=AX.X)

                pnm = scr.tile([128, 4], f32, name="pnm", tag="s4b")
                nc.vector.tensor_scalar(pnm, pn2, 1e-30, None, OP.max)
                pl = scr.tile([128, 4], f32, name="pl", tag="s4c")
                nc.scalar.activation(pl, pnm, ACT.Ln)
                pden = PT(pers, [128, 4], f32, "pden")
                nc.scalar.activation(pden, pl, ACT.Exp, scale=-0.5)
                nc.vector.tensor_tensor(pden, pden, v1, OP.mult)

                pnz = PT(pers, [128, 4, D], f32r, "pnz")
                nc.vector.tensor_tensor(
                    pnz, protos, pden[:, :, None].to_broadcast([128, 4, D]), OP.mult
                )
                dsq = scr1.tile([128, 4, D], f32, name="dsq", tag="sq4")
                nc.vector.tensor_tensor(dsq, pnz, pnz, OP.mult)
                d2 = PT(pers, [128, 4], f32, "d2")
                nc.vector.reduce_sum(d2, dsq, axis=AX.X)

                pnzT = [PT(pers, [128, C], f32r, f"pnzT{d}") for d in range(2)]
                for g in range(2):
                    gt = big_ps(f"gt{g}")
                    for k in range(2):
                        cb = g * 2 + k
                        for d in range(2):
                            nc.tensor.transpose(
                                gt[:, (k * 2 + d) * 512:(k * 2 + d) * 512 + 128].bitcast(f32r),
                                pnz[:, cb, d * 128:(d + 1) * 128],
                                ident_r,
                            )
                    for d in range(2):
                        src = gt.rearrange("p (k x c) -> p k x c", k=2, x=2)[
                            :, :, d, 0:128]
                        dst = pnzT[d].rearrange("p (g k c) -> p g k c", g=2, k=2)[:, g]
                        nc.vector.tensor_copy(dst, src)

                g2 = PT(pers, [128, 4], f32, "g2")
                gp = big_ps("gp")
                for cb in range(4):
                    for d in range(2):
                        nc.tensor.matmul(
                            gp[:, cb * 512:(cb + 1) * 512],
                            pnzT[d][:, cb * 128:(cb + 1) * 128],
                            pnzT[d][:, :],
                            start=(d == 0), stop=(d == 1),
                        )
                for cb in range(4):
                    nc.scalar.activation(
                        gp[:, cb * 512:(cb + 1) * 512],
                        gp[:, cb * 512:(cb + 1) * 512],
                        ACT.Square, accum_out=g2[:, cb:cb + 1],
                    )
                d2sq = scr.tile([128, 4], f32, name="d2sq", tag="s4")
                nc.vector.tensor_tensor(d2sq, d2, d2, OP.mult)
                g2r = scr.tile([128, 1], f32, name="g2r", tag="rst")
                nc.vector.reduce_sum(g2r, g2, axis=AX.X)
                d2r = scr.tile([128, 1], f32, name="d2r", tag="rst")
                nc.vector.reduce_sum(d2r, d2sq, axis=AX.X)
                nc.vector.tensor_tensor(finals[:, 4:5], g2r, d2r, OP.subtract)
                nc.vector.reduce_sum(finals[:, 5:6], v1, axis=AX.X)

            def compute_rows(j, r0, r1):
                p = j % 2
                ztf2 = bufs(p)["ztf2"]
                for r in range(r0, r1):
                    lhsT = ztf2[:, :, r * 128:(r + 1) * 128]
                    for jc in range(4):
                        ps = big_ps("sim")
                        for jb in range(4):
                            c0 = jc * 2048 + jb * 512
                            nc.tensor.matmul(
                                ps[:, jb * 512:(jb + 1) * 512],
                                lhsT,
                                ztf2[:, :, c0:c0 + 512],
                                start=True, stop=True, perf_mode=DR,
                            )
                        if jc == 0:
                            nc.vector.tensor_tensor(
                                ps[:, r * 128:(r + 1) * 128],
                                ps[:, r * 128:(r + 1) * 128], onemI, OP.mult,
                            )
                        if (r, jc) in DVE_BLOCKS:
                            exd = scr2.tile([128, 2048], i16, name="exd", tag="exd")
                            nc.vector.tensor_scalar(
                                exd, ps, SCH_A, SCH_B, OP.mult, OP.add
                            )
                            nc.vector.reduce_sum(
                                rsAll[:, r, jc:jc + 1], exd.bitcast(bf16), axis=AX.X
                            )
                        else:
                            nc.scalar.activation(
                                ps, ps, ACT.Exp, scale=INV_TAU / 256.0,
                                accum_out=rsAll[:, r, jc:jc + 1],
                            )

            def tail(j):
                p = j % 2
                bb = bufs(p)
                O8, ssqs, ce_sums, gls = (
                    bb["O8"], bb["ssqs"], bb["ce_sums"], bb["gls"])
                nc.vector.reduce_sum(rowsums, rsAll, axis=AX.X)
                nc.vector.tensor_scalar(rowsums, rowsums, -1.0, None, OP.add)

                lse = PT(pers, [128, T], f32r, "lse")
                nc.scalar.activation(lse, rowsums, ACT.Ln)
                lse_ce = PT(pers, [128, T], f32, "lse_ce")
                nc.scalar.activation(lse_ce, ce_sums, ACT.Ln)

                ced = scr.tile([128, T], f32, name="ced", tag="ced")
                nc.vector.tensor_tensor(ced, lse_ce, gls, OP.subtract)
                celoc = PT(pers, [128, 1], f32, "celoc")
                nc.vector.reduce_sum(celoc, ced, axis=AX.X)
                sseloc = PT(pers, [128, 1], f32, "sseloc")
                nc.vector.reduce_sum(sseloc, ssqs, axis=AX.X)

                lse2 = PT(pers, [128, T, 2], f32r, "lse2")
                if j == 0:
                    nc.vector.tensor_copy(
                        lse2[:, :, 1:2],
                        ones_c[:, 0:1, None].to_broadcast([128, T, 1]),
                    )
                nc.vector.tensor_copy(lse2[:, :, 0:1], lse[:, :, None])
                lsetile = big_ps("lse")
                for t in range(T):
                    for cb in range(4):
                        nc.tensor.matmul(
                            lsetile[:, cb * 512:cb * 512 + 2],
                            O8[:, t, cb * 128:(cb + 1) * 128],
                            lse2[:, t, :],
                            start=(t == 0), stop=(t == T - 1),
                        )
                lsS = PT(pers, [128, 4], f32, "lsS")
                nc.vector.tensor_copy(
                    lsS[:, :, None],
                    lsetile.rearrange("p (c x) -> p c x", c=4)[:, :, 0:1],
                )
                v2p = PT(pers, [128, 4], f32, "v2")
                nc.vector.tensor_tensor(lsS, lsS, v2p, OP.mult)
                nc.vector.reduce_sum(finals[:, 1:2], lsS, axis=AX.X)

                nc.vector.tensor_copy(finals[:, 6:7], celoc)
                nc.vector.tensor_copy(finals[:, 7:8], sseloc)

                nc.sync.dma_start(out_losses[:, :], finals)

            # ---------- software-pipelined emission ----------
            prep_head(0)
            prep_znorm(0)
            prep_gather(0)
            prep_seg(0)
            prep_ce(0)
            prep_loads(0)
            for j in range(unroll):
                n = j + 1
                if n < unroll:
                    prep_head(n)
                compute_rows(j, 0, 2)
                if n < unroll:
                    prep_znorm(n)
                    prep_gather(n)
                compute_rows(j, 2, 4)
                if n < unroll:
                    prep_seg(n)
                    prep_ce(n)
                proto_block(j)
                compute_rows(j, 4, 8)
                tail(j)
                if n < unroll:
                    prep_loads(n)

    nc.compile()
    return nc


def _get_nc():
    if "nc" not in _CACHE:
        _CACHE["nc"] = _build()
    return _CACHE["nc"]


def kernel(logits, embeddings, labels):
    from concourse import bass_utils

    nc = _get_nc()

    logits = np.ascontiguousarray(np.asarray(logits, dtype=np.float32))
    embeddings = np.ascontiguousarray(np.asarray(embeddings, dtype=np.float32))
    labels_np = np.asarray(labels)

    in_maps = []
    for c in range(NCORES):
        sl = slice(c * SH, (c + 1) * SH)
        lab_f = labels_np[sl].astype(np.float32).reshape(T, 128).T
        in_maps.append({
            "logits": logits[sl],
            "emb": embeddings[sl],
            "labels_f": np.ascontiguousarray(lab_f),
        })

    res = bass_utils.run_bass_kernel_spmd(nc, in_maps, core_ids=list(range(NCORES)))

    p0 = res.results[0]["partials"].astype(np.float64)
    t3a = p0[:, 0].sum()
    nvalid = p0[:, 2].sum()
    cntpn2 = p0[:, 3].sum()
    l4num = p0[:, 4].sum()
    npres = p0[:, 5].sum()
    t3b = ce = sse = 0.0
    for c in range(NCORES):
        pc = res.results[c]["partials"].astype(np.float64)
        t3b += pc[:, 1].sum()
        ce += pc[:, 6].sum()
        sse += pc[:, 7].sum()

    l1 = ce / B
    l2 = (sse - cntpn2) / B
    l3 = -(t3a - t3b) / max(nvalid, 1.0)
    l4 = l4num / max(npres * npres - npres, 1.0)
    total = l1 + ALPHA * l2 + BETA * l3 + GAMMA * l4
    return tuple(np.float32(v) for v in (total, l1, l2, l3, l4))


# revision 10
# speedup vs baseline: 1.2617x; 1.0042x over previous
# Trainium2 Bass kernel for nn_CombinedLoss — v6
#
# v6 = v4 (single rotating PSUM pool of 2x[128,2048], fp8 DoubleRow sim
# matmuls, ACT/DVE exp split, batched prep ops) + SOFTWARE-PIPELINED EMISSION:
# the unrolled loop emits phase A of iteration k+1 BEFORE phase B/C of
# iteration k, with parity-double-buffered ez8/O8/ztf2 (+ small per-iteration
# scalars). Each engine's instruction stream then flows without stalling on
# the transpose->AllGather->load backbone: prep(k+1) work fills the gaps
# while iteration k's sim/exp pipeline runs.
#
# Output: partials per core, reduced on host exactly like the baseline.

import numpy as np

B = 8192
C = 512
D = 256
NCORES = 8
SH = B // NCORES
T = SH // 128
ALPHA = 0.5
BETA = 0.5
GAMMA = 0.5
INV_TAU = 10.0
EPS = 1e-8
UNROLL = 128

SCH_A = 184.6649652337873 * (INV_TAU / 256.0)
SCH_B = 16248.78071298956

# sim (r, jc) 2048-col blocks on DVE (Schraudolph); jc==0 holds the diagonal.
DVE_BLOCKS = {(r, 3) for r in range(6)}

_CACHE = {}


def _build(unroll=UNROLL):
    import concourse.bass as bass
    import concourse.mybir as mybir
    import concourse.tile as tile
    from concourse import bacc
    from concourse.masks import make_identity

    f32 = mybir.dt.float32
    f32r = mybir.dt.float32r
    bf16 = mybir.dt.bfloat16
    f8 = mybir.dt.float8e4
    i16 = mybir.dt.int16
    i32 = mybir.dt.int32
    AX = mybir.AxisListType
    OP = mybir.AluOpType
    ACT = mybir.ActivationFunctionType
    DR = mybir.MatmulPerfMode.DoubleRow

    nc = bacc.Bacc("TRN2", target_bir_lowering=False, debug=False, num_devices=NCORES)

    lg_in = nc.dram_tensor("logits", [SH, C], f32, kind="ExternalInput")
    em_in = nc.dram_tensor("emb", [SH, D], f32r, kind="ExternalInput")
    lab_in = nc.dram_tensor("labels_f", [128, T], f32, kind="ExternalInput")
    out_losses = nc.dram_tensor("partials", [128, 8], f32, kind="ExternalOutput")

    with tile.TileContext(nc) as tc:
        with (
            tc.tile_pool(name="const", bufs=1) as constp,
            tc.tile_pool(name="persist", bufs=1) as pers,
            tc.tile_pool(name="scratch", bufs=3) as scr,
            tc.tile_pool(name="scr1", bufs=1) as scr1,
            tc.tile_pool(name="scr2", bufs=2) as scr2,
            tc.tile_pool(name="psum8", bufs=2, space="PSUM") as psp,
            tc.tile_pool(name="dram", bufs=1, space="DRAM") as dram,
        ):
            _tiles = {}

            def PT(pool, shape, dtype, name):
                if name not in _tiles:
                    _tiles[name] = pool.tile(shape, dtype, name=name)
                return _tiles[name]

            def big_ps(name):
                return psp.tile([128, 2048], f32, name=name, tag="big")

            # ---------- constants ----------
            ident = constp.tile([128, 128], f32, name="ident")
            make_identity(nc, ident)
            ident_r = constp.tile([128, 128], f32r, name="ident_r")
            nc.vector.tensor_copy(ident_r, ident)
            ones_c = constp.tile([128, 1], f32, name="ones_c")
            nc.vector.memset(ones_c, 1.0)
            ones2 = constp.tile([128, 2], f32, name="ones2")
            nc.vector.memset(ones2, 1.0)
            ones_r2 = constp.tile([128, 2], f32r, name="ones_r2")
            nc.vector.tensor_copy(ones_r2, ones2)
            onemI = constp.tile([128, 128], f32, name="onemI")
            nc.vector.memset(onemI, 1.0)
            nc.gpsimd.affine_select(
                out=onemI, in_=onemI, compare_op=OP.not_equal, fill=0.0,
                base=0, pattern=[[-1, 128]], channel_multiplier=1,
            )
            iota_i = constp.tile([128, C], i32, name="iota_i")
            nc.gpsimd.iota(iota_i, pattern=[[1, C]], base=0, channel_multiplier=0)
            iota_f = constp.tile([128, C], f32, name="iota_f")
            nc.vector.tensor_copy(iota_f, iota_i)

            lab = constp.tile([128, T], f32, name="lab")
            nc.sync.dma_start(lab, lab_in[:, :])

            pid = nc.sync.partition_id()

            # parity-indexed persistent buffers (iteration j uses p = j % 2)
            def bufs(p):
                return dict(
                    ez8=PT(pers, [128, T, 513], f32r, f"ez8_{p}"),
                    O8=PT(pers, [128, T, C], f32r, f"O8_{p}"),
                    ztf2=PT(pers, [128, 2, B], f8, f"ztf2_{p}"),
                    ssqs=PT(pers, [128, T], f32, f"ssqs_{p}"),
                    zden=PT(pers, [128, T], f32, f"zden_{p}"),
                    ce_sums=PT(pers, [128, T], f32, f"ce_sums_{p}"),
                    gls=PT(pers, [128, T], f32, f"gls_{p}"),
                )

            zts = [PT(pers, [128, SH], f8, f"zts{d}") for d in range(2)]
            seg_sb = PT(pers, [128, 4, 513], bf16, "seg_sb")
            rsAll = PT(pers, [128, T, 4], f32, "rsAll")
            rowsums = PT(pers, [128, T], f32, "rowsums")
            finals = PT(pers, [128, 8], f32, "finals")

            dramb = {}

            def prep_head(j):
                """DRAM scratch + embeddings DMA + row sumsq for iteration j."""
                p = j % 2
                bb = bufs(p)
                ez8, ssqs = bb["ez8"], bb["ssqs"]

                zt_local = dram.tile([D, SH], f8, name=f"zt_local{j}")
                zt_gath = dram.tile(
                    [NCORES, D, SH], f8, name=f"zt_gath{j}", addr_space="Shared"
                )
                seg_in = dram.tile([128, 4, 513], bf16, name=f"seg_in{j}")
                seg_out = dram.tile(
                    [128, 4, 513], bf16, name=f"seg_out{j}", addr_space="Shared"
                )
                dramb[j] = (zt_local, zt_gath, seg_in, seg_out)

                if j < 2:
                    nc.vector.tensor_copy(
                        ez8[:, :, 512:513],
                        ones_c[:, 0:1, None].to_broadcast([128, T, 1]),
                    )
                nc.sync.dma_start(
                    ez8[:, :, 0:D], em_in.rearrange("(t p) d -> p t d", p=128)
                )
                for t in range(T):
                    sq = scr.tile([128, D], f32, name="sq", tag="sq")
                    nc.vector.scalar_tensor_tensor(
                        out=sq, in0=ez8[:, t, 0:D], scalar=1.0, in1=ez8[:, t, 0:D],
                        op0=OP.mult, op1=OP.mult, accum_out=ssqs[:, t:t + 1],
                    )

            def prep_znorm(j):
                p = j % 2
                bb = bufs(p)
                ez8, O8, ssqs, zden = bb["ez8"], bb["O8"], bb["ssqs"], bb["zden"]
                zl = scr.tile([128, T], f32, name="zl", tag="zl")
                nc.scalar.activation(zl, ssqs, ACT.Ln)
                nc.scalar.activation(zden, zl, ACT.Exp, scale=-0.5)
                nc.vector.tensor_tensor(
                    ez8[:, :, D:2 * D], ez8[:, :, 0:D],
                    zden[:, :, None].to_broadcast([128, T, D]), OP.mult,
                )
                for t in range(T):
                    nc.vector.tensor_scalar(
                        O8[:, t, :], iota_f, lab[:, t:t + 1], None, OP.is_equal
                    )

            def prep_gather(j):
                p = j % 2
                ez8 = bufs(p)["ez8"]
                zt_local, zt_gath, _, _ = dramb[j]
                for d in range(2):
                    for g in range(2):
                        ptile = big_ps(f"tr{d}{g}")
                        for k in range(4):
                            t = g * 4 + k
                            nc.tensor.transpose(
                                ptile[:, k * 512:k * 512 + 128].bitcast(f32r),
                                ez8[:, t, D + d * 128:D + (d + 1) * 128],
                                ident_r,
                            )
                        src = ptile.rearrange("p (k c) -> p k c", k=4)[:, :, 0:128]
                        dst = zts[d].rearrange("p (g k c) -> p g k c", g=2, k=4)[:, g]
                        nc.vector.tensor_scalar(dst, src, 16.0, None, OP.mult)
                for d in range(2):
                    nc.sync.dma_start(zt_local[d * 128:(d + 1) * 128, :], zts[d])
                nc.gpsimd.collective_compute(
                    "AllGather", OP.bypass,
                    replica_groups=[list(range(NCORES))],
                    ins=[zt_local.opt()], outs=[zt_gath.opt()],
                )

            def prep_seg(j):
                p = j % 2
                bb = bufs(p)
                ez8, O8 = bb["ez8"], bb["O8"]
                _, _, seg_in, seg_out = dramb[j]
                for half in range(2):
                    stile = big_ps(f"seg{half}")
                    for t in range(T):
                        for k in range(2):
                            cb = half * 2 + k
                            lhs = O8[:, t, cb * 128:(cb + 1) * 128]
                            nc.tensor.matmul(
                                stile[:, k * 1024:k * 1024 + 512], lhs,
                                ez8[:, t, 0:512],
                                start=(t == 0), stop=(t == T - 1),
                            )
                            nc.tensor.matmul(
                                stile[:, k * 1024 + 512:k * 1024 + 514], lhs, ones_r2,
                                start=(t == 0), stop=(t == T - 1),
                            )
                    nc.vector.tensor_copy(
                        seg_sb[:, half * 2:half * 2 + 2, :],
                        stile.rearrange("p (k c) -> p k c", k=2)[:, :, 0:513],
                    )
                nc.sync.dma_start(seg_in[:, :, :], seg_sb)
                nc.gpsimd.collective_compute(
                    "AllReduce", OP.add,
                    replica_groups=[list(range(NCORES))],
                    ins=[seg_in.opt()], outs=[seg_out.opt()],
                )

            def prep_ce(j):
                p = j % 2
                bb = bufs(p)
                O8, ce_sums, gls = bb["O8"], bb["ce_sums"], bb["gls"]
                lgt8 = scr1.tile([128, T, C], f32, name="lgt8", tag="lgt8")
                nc.sync.dma_start(lgt8, lg_in.rearrange("(t p) d -> p t d", p=128))
                for t in range(T):
                    gsc = scr.tile([128, C], f32, name="gsc", tag="gsc")
                    nc.vector.scalar_tensor_tensor(
                        out=gsc, in0=O8[:, t, :], scalar=1.0, in1=lgt8[:, t, :],
                        op0=OP.mult, op1=OP.mult, accum_out=gls[:, t:t + 1],
                    )
                for t in range(T):
                    if t >= T - 3:
                        exc = scr.tile([128, C], i16, name="exc", tag="exc")
                        nc.vector.tensor_scalar(
                            exc, lgt8[:, t, :], 184.6649652337873, SCH_B,
                            OP.mult, OP.add,
                        )
                        nc.vector.reduce_sum(
                            ce_sums[:, t:t + 1], exc.bitcast(bf16), axis=AX.X
                        )
                    else:
                        nc.scalar.activation(
                            lgt8[:, t, :], lgt8[:, t, :], ACT.Exp,
                            accum_out=ce_sums[:, t:t + 1],
                        )

            def prep_loads(j):
                """ztf2 loads for iteration j (waits on its AllGather)."""
                p = j % 2
                ztf2 = bufs(p)["ztf2"]
                zt_local, zt_gath, _, _ = dramb[j]
                for d in range(2):
                    nc.sync.dma_start(
                        ztf2[:, d, 0:SH], zt_local[d * 128:(d + 1) * 128, :]
                    )
                for blk in range(1, NCORES):
                    src = (pid + blk) % NCORES
                    nc.sync.dma_start(
                        ztf2[:, :, blk * SH:(blk + 1) * SH],
                        zt_gath[bass.ds(src, 1), :, :].rearrange(
                            "x (d p) c -> p (x d) c", p=128),
                    )

            def proto_block(j):
                """AllReduce-dependent class math (emitted mid-B)."""
                _, _, _, seg_out = dramb[j]
                sseg_h = PT(pers, [128, 4, 513], bf16, "sseg_h")
                nc.sync.dma_start(sseg_h, seg_out[:, :, :])
                sseg = PT(pers, [128, 4, 513], f32, "sseg")
                nc.vector.tensor_copy(sseg, sseg_h)

                cnts = PT(pers, [128, 4], f32, "cnts")
                nc.vector.tensor_copy(cnts[:, :, None], sseg[:, :, 512:513])
                cntm = PT(pers, [128, 4], f32, "cntm")
                nc.vector.tensor_scalar(cntm, cnts, 1.0, None, OP.max)
                rcnt = PT(pers, [128, 4], f32, "rcnt")
                nc.vector.reciprocal(rcnt, cntm)
                cm1 = PT(pers, [128, 4], f32, "cm1")
                nc.vector.tensor_scalar(cm1, cnts, -1.0, 1.0, OP.add, OP.max)
                rcm1 = PT(pers, [128, 4], f32, "rcm1")
                nc.vector.reciprocal(rcm1, cm1)
                v2 = PT(pers, [128, 4], f32, "v2")
                nc.vector.tensor_scalar(v2, cnts, 2.0, None, OP.is_ge)
                v1 = PT(pers, [128, 4], f32, "v1")
                nc.vector.tensor_scalar(v1, cnts, 0.5, None, OP.is_ge)

                protos = PT(pers, [128, 4, D], f32, "protos")
                nc.vector.tensor_tensor(
                    protos, sseg[:, :, 0:D],
                    rcnt[:, :, None].to_broadcast([128, 4, D]), OP.mult,
                )
                psq = scr1.tile([128, 4, D], f32, name="psq", tag="sq4")
                nc.vector.tensor_tensor(psq, protos, protos, OP.mult)
                pn2 = PT(pers, [128, 4], f32, "pn2")
                nc.vector.reduce_sum(pn2, psq, axis=AX.X)
                ssq2 = scr1.tile([128, 4, D], f32, name="ssq2", tag="sq4b")
                nc.vector.tensor_tensor(
                    ssq2, sseg[:, :, D:2 * D], sseg[:, :, D:2 * D], OP.mult
                )
                S2 = PT(pers, [128, 4], f32, "S2")
                nc.vector.reduce_sum(S2, ssq2, axis=AX.X)

                t3 = PT(pers, [128, 4], f32, "t3")
                nc.vector.tensor_tensor(t3, S2, cnts, OP.subtract)
                nc.vector.tensor_scalar(t3, t3, INV_TAU, None, OP.mult)
                nc.vector.tensor_tensor(t3, t3, rcm1, OP.mult)
                nc.vector.tensor_tensor(t3, t3, v2, OP.mult)
                nc.vector.reduce_sum(finals[:, 0:1], t3, axis=AX.X)
                nval = scr.tile([128, 4], f32, name="nval", tag="s4")
                nc.vector.tensor_tensor(nval, v2, cnts, OP.mult)
                nc.vector.reduce_sum(finals[:, 2:3], nval, axis=AX.X)
                cpn = scr.tile([128, 4], f32, name="cpn", tag="s4")
                nc.vector.tensor_tensor(cpn, cnts, pn2, OP.mult)
                nc.vector.reduce_sum(finals[:, 3:4], cpn, axis=AX.X)

                pnm = scr.tile([128, 4], f32, name="pnm", tag="s4b")
                nc.vector.tensor_scalar(pnm, pn2, 1e-30, None, OP.max)
                pl = scr.tile([128, 4], f32, name="pl", tag="s4c")
                nc.scalar.activation(pl, pnm, ACT.Ln)
                pden = PT(pers, [128, 4], f32, "pden")
                nc.scalar.activation(pden, pl, ACT.Exp, scale=-0.5)
                nc.vector.tensor_tensor(pden, pden, v1, OP.mult)

                pnz = PT(pers, [128, 4, D], f32r, "pnz")
                nc.vector.tensor_tensor(
                    pnz, protos, pden[:, :, None].to_broadcast([128, 4, D]), OP.mult
                )
                dsq = scr1.tile([128, 4, D], f32, name="dsq", tag="sq4")
                nc.vector.tensor_tensor(dsq, pnz, pnz, OP.mult)
                d2 = PT(pers, [128, 4], f32, "d2")
                nc.vector.reduce_sum(d2, dsq, axis=AX.X)

                pnzT = [PT(pers, [128, C], f32r, f"pnzT{d}") for d in range(2)]
                for g in range(2):
                    gt = big_ps(f"gt{g}")
                    for k in range(2):
                        cb = g * 2 + k
                        for d in range(2):
                            nc.tensor.transpose(
                                gt[:, (k * 2 + d) * 512:(k * 2 + d) * 512 + 128].bitcast(f32r),
                                pnz[:, cb, d * 128:(d + 1) * 128],
                                ident_r,
                            )
                    for d in range(2):
                        src = gt.rearrange("p (k x c) -> p k x c", k=2, x=2)[
                            :, :, d, 0:128]
                        dst = pnzT[d].rearrange("p (g k c) -> p g k c", g=2, k=2)[:, g]
                        nc.vector.tensor_copy(dst, src)

                g2 = PT(pers, [128, 4], f32, "g2")
                gp = big_ps("gp")
                for cb in range(4):
                    for d in range(2):
                        nc.tensor.matmul(
                            gp[:, cb * 512:(cb + 1) * 512],
                            pnzT[d][:, cb * 128:(cb + 1) * 128],
                            pnzT[d][:, :],
                            start=(d == 0), stop=(d == 1),
                        )
                for cb in range(4):
                    nc.scalar.activation(
                        gp[:, cb * 512:(cb + 1) * 512],
                        gp[:, cb * 512:(cb + 1) * 512],
                        ACT.Square, accum_out=g2[:, cb:cb + 1],
                    )
                d2sq = scr.tile([128, 4], f32, name="d2sq", tag="s4")
                nc.vector.tensor_tensor(d2sq, d2, d2, OP.mult)
                g2r = scr.tile([128, 1], f32, name="g2r", tag="rst")
                nc.vector.reduce_sum(g2r, g2, axis=AX.X)
                d2r = scr.tile([128, 1], f32, name="d2r", tag="rst")
                nc.vector.reduce_sum(d2r, d2sq, axis=AX.X)
                nc.vector.tensor_tensor(finals[:, 4:5], g2r, d2r, OP.subtract)
                nc.vector.reduce_sum(finals[:, 5:6], v1, axis=AX.X)

            def compute_rows(j, r0, r1):
                p = j % 2
                ztf2 = bufs(p)["ztf2"]
                for r in range(r0, r1):
                    lhsT = ztf2[:, :, r * 128:(r + 1) * 128]
                    for jc in range(4):
                        ps = big_ps("sim")
                        for jb in range(4):
                            c0 = jc * 2048 + jb * 512
                            nc.tensor.matmul(
                                ps[:, jb * 512:(jb + 1) * 512],
                                lhsT,
                                ztf2[:, :, c0:c0 + 512],
                                start=True, stop=True, perf_mode=DR,
                            )
                        if jc == 0:
                            nc.vector.tensor_tensor(
                                ps[:, r * 128:(r + 1) * 128],
                                ps[:, r * 128:(r + 1) * 128], onemI, OP.mult,
                            )
                        if (r, jc) in DVE_BLOCKS:
                            exd = scr2.tile([128, 2048], i16, name="exd", tag="exd")
                            nc.vector.tensor_scalar(
                                exd, ps, SCH_A, SCH_B, OP.mult, OP.add
                            )
                            nc.vector.reduce_sum(
                                rsAll[:, r, jc:jc + 1], exd.bitcast(bf16), axis=AX.X
                            )
                        else:
                            nc.scalar.activation(
                                ps, ps, ACT.Exp, scale=INV_TAU / 256.0,
                                accum_out=rsAll[:, r, jc:jc + 1],
                            )

            def tail(j):
                p = j % 2
                bb = bufs(p)
                O8, ssqs, ce_sums, gls = (
                    bb["O8"], bb["ssqs"], bb["ce_sums"], bb["gls"])
                nc.vector.reduce_sum(rowsums, rsAll, axis=AX.X)
                nc.vector.tensor_scalar(rowsums, rowsums, -1.0, None, OP.add)

                lse = PT(pers, [128, T], f32r, "lse")
                nc.scalar.activation(lse, rowsums, ACT.Ln)
                lse_ce = PT(pers, [128, T], f32, "lse_ce")
                nc.scalar.activation(lse_ce, ce_sums, ACT.Ln)

                ced = scr.tile([128, T], f32, name="ced", tag="ced")
                nc.vector.tensor_tensor(ced, lse_ce, gls, OP.subtract)
                celoc = PT(pers, [128, 1], f32, "celoc")
                nc.vector.reduce_sum(celoc, ced, axis=AX.X)
                sseloc = PT(pers, [128, 1], f32, "sseloc")
                nc.vector.reduce_sum(sseloc, ssqs, axis=AX.X)

                lse2 = PT(pers, [128, T, 2], f32r, "lse2")
                if j == 0:
                    nc.vector.tensor_copy(
                        lse2[:, :, 1:2],
                        ones_c[:, 0:1, None].to_broadcast([128, T, 1]),
                    )
                nc.vector.tensor_copy(lse2[:, :, 0:1], lse[:, :, None])
                lsetile = big_ps("lse")
                for t in range(T):
                    for cb in range(4):
                        nc.tensor.matmul(
                            lsetile[:, cb * 512:cb * 512 + 2],
                            O8[:, t, cb * 128:(cb + 1) * 128],
                            lse2[:, t, :],
                            start=(t == 0), stop=(t == T - 1),
                        )
                lsS = PT(pers, [128, 4], f32, "lsS")
                nc.vector.tensor_copy(
                    lsS[:, :, None],
                    lsetile.rearrange("p (c x) -> p c x", c=4)[:, :, 0:1],
                )
                v2p = PT(pers, [128, 4], f32, "v2")
                nc.vector.tensor_tensor(lsS, lsS, v2p, OP.mult)
                nc.vector.reduce_sum(finals[:, 1:2], lsS, axis=AX.X)

                nc.vector.tensor_copy(finals[:, 6:7], celoc)
                nc.vector.tensor_copy(finals[:, 7:8], sseloc)

                nc.sync.dma_start(out_losses[:, :], finals)

            # ---------- software-pipelined emission ----------
            prep_head(0)
            prep_znorm(0)
            prep_gather(0)
            prep_seg(0)
            prep_ce(0)
            prep_loads(0)
            for j in range(unroll):
                n = j + 1
                if n < unroll:
                    prep_head(n)
                compute_rows(j, 0, 2)
                if n < unroll:
                    prep_znorm(n)
                    prep_gather(n)
                compute_rows(j, 2, 4)
                if n < unroll:
                    prep_seg(n)
                    prep_ce(n)
                proto_block(j)
                compute_rows(j, 4, 8)
                if n < unroll:
                    prep_loads(n)
                tail(j)

    nc.compile()
    return nc


def _get_nc():
    if "nc" not in _CACHE:
        _CACHE["nc"] = _build()
    return _CACHE["nc"]


def kernel(logits, embeddings, labels):
    from concourse import bass_utils

    nc = _get_nc()

    logits = np.ascontiguousarray(np.asarray(logits, dtype=np.float32))
    embeddings = np.ascontiguousarray(np.asarray(embeddings, dtype=np.float32))
    labels_np = np.asarray(labels)

    in_maps = []
    for c in range(NCORES):
        sl = slice(c * SH, (c + 1) * SH)
        lab_f = labels_np[sl].astype(np.float32).reshape(T, 128).T
        in_maps.append({
            "logits": logits[sl],
            "emb": embeddings[sl],
            "labels_f": np.ascontiguousarray(lab_f),
        })

    res = bass_utils.run_bass_kernel_spmd(nc, in_maps, core_ids=list(range(NCORES)))

    p0 = res.results[0]["partials"].astype(np.float64)
    t3a = p0[:, 0].sum()
    nvalid = p0[:, 2].sum()
    cntpn2 = p0[:, 3].sum()
    l4num = p0[:, 4].sum()
    npres = p0[:, 5].sum()
    t3b = ce = sse = 0.0
    for c in range(NCORES):
        pc = res.results[c]["partials"].astype(np.float64)
        t3b += pc[:, 1].sum()
        ce += pc[:, 6].sum()
        sse += pc[:, 7].sum()

    l1 = ce / B
    l2 = (sse - cntpn2) / B
    l3 = -(t3a - t3b) / max(nvalid, 1.0)
    l4 = l4num / max(npres * npres - npres, 1.0)
    total = l1 + ALPHA * l2 + BETA * l3 + GAMMA * l4
    return tuple(np.float32(v) for v in (total, l1, l2, l3, l4))
